# revision 38
# baseline (speedup 1.0000x reference)
"""Multi-head self-attention kernel for 8 Trainium2 NeuronCores.

Sharding: core c = (b, g) with b = batch index (4), g = head-group (2).
Each core computes attention for one batch element and 8 of the 16 heads,
including its slice of the QKV projections and a partial out-projection
(Y_partial = O_heads @ Wo[rows of its heads]).  The host sums the two
head-group partials per batch, transposes (the device produces Y^T), and
adds bo.

On-device layout is fully "transposed": x^T [D, S] in (bf16), Q^T/K^T
[dk, S] fp8e4m3, scores S^T = K_h Q_h^T [k, q] (softmax along partitions
via a ones-column appended to V: the PV matmul yields the softmax
denominator in its last row), output Y^T [D, S] fp32.

Precision plan (rel-err budget, gate 2e-2; measured ~1.4e-2 in numpy):
  - x, Wq/Wk/Wv/Wo, out-projection: bf16 inputs, fp32 PSUM accum.
  - Q^T/K^T and scores matmul: fp8e4m3 (normal mode, contraction 64).
  - P = exp(scores) and V: fp8e4m3; PV matmul in DoubleRow perf mode
    (2 key-chunks per instruction, 0.5 cycles/row) by pairing keys
    (k, k+128) within each partition: exp of chunk 2c -> p8[:,0,:],
    chunk 2c+1 -> p8[:,1,:]; V stored [P, KC, H, 65] so the stationary
    slice [:, 2c:2c+2, h, :] pairs identically.
  - exp: ACT (exact, scale folded) for most chunks; a tunable subset on
    DVE via a Schraudolph bit-trick straight to fp8 bits.
"""

import sys

sys.path.insert(0, "/opt/trn_rl_repo")

from contextlib import ExitStack

import numpy as np
import ml_dtypes

import concourse.bass as bass
import concourse.tile as tile
from concourse import bacc, mybir
from concourse.bass_utils import run_bass_kernel_spmd

F32 = mybir.dt.float32
F32R = mybir.dt.float32r
BF16 = mybir.dt.bfloat16
FP8 = mybir.dt.float8e4
U8 = mybir.dt.uint8
P = 128  # SBUF partitions

D_MODEL = 1024
NHEAD = 16
DK = D_MODEL // NHEAD  # 64
BATCH = 4
SEQ = 2048
N_CORES = 8
HL = NHEAD // 2  # heads per core (head-group of 8)

LN2 = float(np.log(2.0))
# DVE Schraudolph exp -> fp8e4m3 bits: i = trunc(s*A + B), bitcast u8->fp8.
# fp8 bias 7, 3 mantissa bits: value ~= 2^((i-56)/8).  Want exp(s/8) =
# 2^(s/(8*ln2)) -> A = 1/ln2.  B = 56 - 0.0573*8 (sawtooth centering)
# + 0.5 (truncation -> round).
SCH_A = 1.0 / LN2
SCH_B = 56.0 - 0.0573 * 8.0 + 0.5


def build_bass(D=D_MODEL, S=SEQ, HLOC=HL, QB=512, repeat=1,
               qtpb=4, pexpb=6, psab=1, psaccb=3, sch_set=(2, 5, 7),
               resb=2):
    """Build the per-core Bass program (same program on all 8 cores)."""
    DC = D // P           # d_model chunks (contraction for projections)
    KC = S // P           # key chunks
    KC2 = KC // 2         # key double-chunks (DoubleRow)
    NQB = S // QB         # q blocks
    NPAIR = HLOC // 2     # head pairs
    HD = HLOC * DK        # local head dim total (512)
    VW = DK + 1           # V columns per head incl. ones column
    NOC = D // P          # out-dim chunks
    EXP_SCALE = 1.0 / np.sqrt(DK)
    cfg = dict(D=D, S=S, HLOC=HLOC, QB=QB, DC=DC, KC=KC, KC2=KC2, NQB=NQB,
               NPAIR=NPAIR, HD=HD, VW=VW, NOC=NOC, EXP_SCALE=EXP_SCALE,
               sch_set=frozenset(sch_set))

    nc = bacc.Bacc("TRN2", target_bir_lowering=False, debug=False,
                   num_devices=N_CORES)

    xT = nc.dram_tensor("xT", [D, S], BF16, kind="ExternalInput")
    Wq = nc.dram_tensor("Wq", [D, HD], BF16, kind="ExternalInput")
    Wk = nc.dram_tensor("Wk", [D, HD], BF16, kind="ExternalInput")
    Wv = nc.dram_tensor("Wv", [D, HD], BF16, kind="ExternalInput")
    Wo = nc.dram_tensor("Wo", [HD, D], BF16, kind="ExternalInput")
    bq_t = nc.dram_tensor("bq_t", [P, NPAIR], F32, kind="ExternalInput")
    bk_t = nc.dram_tensor("bk_t", [P, NPAIR], F32, kind="ExternalInput")
    YT = nc.dram_tensor("YT", [D, S], F32, kind="ExternalOutput")
    dram = dict(xT=xT, Wq=Wq, Wk=Wk, Wv=Wv, Wo=Wo, bq_t=bq_t, bk_t=bk_t,
                YT=YT)

    with tile.TileContext(nc) as tc, ExitStack() as ctx:
        consts = ctx.enter_context(tc.tile_pool(name="consts", bufs=1))
        ktv = ctx.enter_context(tc.tile_pool(name="ktv", bufs=resb))
        wper = ctx.enter_context(tc.tile_pool(name="wper", bufs=1))
        xres = ctx.enter_context(tc.tile_pool(name="xres", bufs=resb))
        ps_a = ctx.enter_context(tc.tile_pool(name="ps_a", bufs=psab, space="PSUM"))
        ps_b = ctx.enter_context(tc.tile_pool(name="ps_b", bufs=1, space="PSUM"))
        ps_acc = ctx.enter_context(tc.tile_pool(name="ps_acc", bufs=psaccb,
                                                space="PSUM"))

        # ---- constants ----
        bq_sb = consts.tile([P, NPAIR], F32, tag="bq")
        bk_sb = consts.tile([P, NPAIR], F32, tag="bk")
        nc.sync.dma_start(bq_sb[:], bq_t.ap())
        nc.sync.dma_start(bk_sb[:], bk_t.ap())

        # warm the ACT exp table early
        warm = consts.tile([1, 2], F32, tag="warm")
        nc.gpsimd.memset(warm[0:1, 0:1], 0.0)
        nc.scalar.activation(warm[0:1, 1:2], warm[0:1, 0:1],
                             mybir.ActivationFunctionType.Exp)

        # weights are constant across repeat bodies: load once
        DC_ = D // P
        HD_ = HLOC * DK

        def load_w(name, d, cols):
            t = wper.tile([P, DC_ * cols], BF16, tag=name, name=name)
            nc.sync.dma_start(
                t[:].rearrange("p (c n) -> p c n", c=DC_),
                d.ap().rearrange("(c p) n -> p c n", p=P))
            return t

        wk_sb = load_w("wk", Wk, HD_)
        wv_sb = load_w("wv", Wv, HD_)
        wq_sb = load_w("wq", Wq, HD_)
        NPAIR_ = HLOC // 2
        wo_sb = wper.tile([P, NPAIR_ * D], BF16, tag="wo", name="wo")
        nc.sync.dma_start(
            wo_sb[:].rearrange("p (r n) -> p r n", r=NPAIR_),
            Wo.ap().rearrange("(r p) n -> p r n", p=P))

        sbs = dict(bq=bq_sb, bk=bk_sb, wk=wk_sb, wv=wv_sb,
                   wq=wq_sb, wo=wo_sb)
        pools = dict(consts=consts, ktv=ktv, wper=wper, xres=xres,
                     ps_a=ps_a, ps_b=ps_b, ps_acc=ps_acc)

        for _rep in range(repeat):
            emit_body(nc, tc, cfg, dram, sbs, pools, qtpb=qtpb, pexpb=pexpb)

    nc.compile()
    return nc


def emit_body(nc, tc, cfg, dram, sbs, pools, qtpb=3, pexpb=3):
    D, S, HLOC, QB = cfg["D"], cfg["S"], cfg["HLOC"], cfg["QB"]
    DC, KC, KC2 = cfg["DC"], cfg["KC"], cfg["KC2"]
    NQB, NPAIR = cfg["NQB"], cfg["NPAIR"]
    HD, VW, NOC, EXP_SCALE = cfg["HD"], cfg["VW"], cfg["NOC"], cfg["EXP_SCALE"]
    sch_set = cfg["sch_set"]
    ktv, wper, xres = pools["ktv"], pools["wper"], pools["xres"]
    ps_a, ps_b, ps_acc = pools["ps_a"], pools["ps_b"], pools["ps_acc"]
    bq_sb, bk_sb = sbs["bq"], sbs["bk"]

    xt_dram3 = dram["xT"].ap().rearrange("(c p) s -> p c s", p=P)
    yt_dram3 = dram["YT"].ap().rearrange("(n p) s -> p n s", p=P)

    # resident tensors
    kt_tiles = [ktv.tile([P, S], FP8, tag=f"kt{p_}", name=f"kt{p_}")
                for p_ in range(NPAIR)]
    # V in fp8, layout [P, KC2, HLOC, 2, 128]: each head's key-chunk pair
    # (2c, 2c+1) at stride 128 (walrus requires the DoubleRow k-tile pair
    # stride in the weights AP to be 64 or 128); only cols 0:VW are used.
    VWP = 128
    v4 = ktv.tile([P, KC2 * HLOC * 2 * VWP], FP8, tag="v4", name="v4")
    v4d = v4[:].rearrange("p (k h t w) -> p k h t w", k=KC2, h=HLOC, t=2)
    xt = xres.tile([P, DC * S], BF16, tag="xt", name="xt")
    xt3 = xt[:].rearrange("p (c s) -> p c s", c=DC)

    def wslice(wt, c, lo, hi):
        return wt[:, c * HD + lo: c * HD + hi]

    wk_sb, wv_sb = sbs["wk"], sbs["wv"]
    wq_sb, wo_sb = sbs["wq"], sbs["wo"]

    # ones columns of V (exactly representable in fp8)
    nc.vector.memset(v4d[:, :, :, :, DK:VW], 1.0)

    # ---- phase 1: K^T and V ----
    for w in range(NQB):
        sl = bass.ts(w, QB)
        nc.sync.dma_start(xt3[:, :, sl], xt_dram3[:, :, sl])

    if True:
        for w in range(NQB):
            sl = bass.ts(w, QB)
            for pr in range(NPAIR):
                kps = ps_b.tile([P, QB], F32, tag="sp", name="kps")
                for c in range(DC):
                    nc.tensor.matmul(kps[:],
                                     wslice(wk_sb, c, pr * P, (pr + 1) * P),
                                     xt3[:, c, sl],
                                     start=(c == 0), stop=(c == DC - 1))
                nc.vector.tensor_scalar_add(kt_tiles[pr][:, sl], kps[:],
                                            bk_sb[:, pr:pr + 1])
            for s4 in range(QB // P):
                k = w * (QB // P) + s4
                vps = ps_b.tile([P, HD], F32, tag="sp", name="vps")
                for c in range(DC):
                    nc.tensor.matmul(vps[:],
                                     xt3[:, c, bass.ts(k, P)],
                                     wslice(wv_sb, c, 0, HD),
                                     start=(c == 0), stop=(c == DC - 1))
                nc.vector.tensor_add(v4d[:, k // 2, :, k % 2, 0:DK],
                                     vps[:].rearrange("p (h d) -> p h d", h=HLOC),
                                     bv_sb[:].rearrange("p (h d) -> p h d", h=HLOC))

    # ---- phase 2: Q^T just-in-time + attention + out-projection ----
    with tc.tile_pool(name="qtp", bufs=qtpb) as qtp, \
         tc.tile_pool(name="pexp", bufs=pexpb) as pexp, \
         tc.tile_pool(name="otp", bufs=NPAIR + 1) as otp, \
         tc.tile_pool(name="misc", bufs=2) as misc:

        def qt_proj(qb, pr):
            qsl = bass.ts(qb, QB)
            qps = ps_b.tile([P, QB], F32, tag="sp", name="qps")
            for c in range(DC):
                nc.tensor.matmul(qps[:],
                                 wslice(wq_sb, c, pr * P, (pr + 1) * P),
                                 xt3[:, c, qsl],
                                 start=(c == 0), stop=(c == DC - 1))
            qt = qtp.tile([P, QB], FP8, tag="qt", name="qt")
            nc.vector.tensor_scalar_add(qt[:], qps[:], bq_sb[:, pr:pr + 1])
            return qt

        pend = [qt_proj(0, 0), qt_proj(0, 1)]
        for qb in range(NQB):
            ot_tiles = []
            for pr in range(NPAIR):
                qt = pend.pop(0)
                kt = kt_tiles[pr]
                # pre-project 2 ahead so the projection chain stays off
                # the critical path.
                nxt = (qb * NPAIR + pr + 2)
                if nxt < NQB * NPAIR:
                    pend.append(qt_proj(nxt // NPAIR, nxt % NPAIR))

                oa = ps_acc.tile([VW, QB], F32, tag="acc", name="oa")
                ob = ps_acc.tile([VW, QB], F32, tag="acc", name="ob")
                accs = (oa, ob)
                for k2 in range(KC2):
                    for h01 in (0, 1):
                        hb = h01 * DK   # head base partition in kt/qt
                        h = 2 * pr + h01
                        se = ps_a.tile([P, QB], F32, tag=f"se{h01}",
                                       name="se")
                        so = ps_a.tile([P, QB], F32, tag=f"so{h01}",
                                       name="so")
                        nc.tensor.matmul(se[:], kt[hb:hb + DK, bass.ts(2 * k2, P)],
                                         qt[hb:hb + DK, :], start=True, stop=True)
                        nc.tensor.matmul(so[:], kt[hb:hb + DK, bass.ts(2 * k2 + 1, P)],
                                         qt[hb:hb + DK, :], start=True, stop=True)
                        p8 = pexp.tile([P, 2 * QB], FP8, tag=f"p8{h01}",
                                       name="p8")
                        p83 = p8[:].rearrange("p (t q) -> p t q", t=2)
                        if (2 * k2 + h01) % 8 in sch_set:
                            u83 = p8[:].bitcast(U8).rearrange(
                                "p (t q) -> p t q", t=2)
                            nc.vector.tensor_scalar(
                                u83[:, 0, :], se[:], SCH_A, SCH_B,
                                mybir.AluOpType.mult, mybir.AluOpType.add)
                            nc.vector.tensor_scalar(
                                u83[:, 1, :], so[:], SCH_A, SCH_B,
                                mybir.AluOpType.mult, mybir.AluOpType.add)
                        else:
                            nc.scalar.activation(p83[:, 0, :], se[:],
                                                 mybir.ActivationFunctionType.Exp,
                                                 scale=float(EXP_SCALE))
                            nc.scalar.activation(p83[:, 1, :], so[:],
                                                 mybir.ActivationFunctionType.Exp,
                                                 scale=float(EXP_SCALE))
                        nc.tensor.matmul(
                            accs[h01][:],
                            v4d[:, k2, h, :, 0:VW],
                            p83[:, :, :],
                            start=(k2 == 0), stop=(k2 == KC2 - 1),
                            perf_mode=mybir.MatmulPerfMode.DoubleRow)

                # normalize rows 0:DK by row DK (the ones-column sums)
                ra = misc.tile([1, QB], F32, tag="ra", name="ra", bufs=1)
                rb = misc.tile([1, QB], F32, tag="rb", name="rb", bufs=1)
                nc.vector.reciprocal(ra[:], oa[DK:VW, :])
                nc.vector.reciprocal(rb[:], ob[DK:VW, :])
                bc = misc.tile([P, QB], F32, tag="bc", name="bc")
                bc2 = misc.tile([P, QB], F32, tag="bc", name="bc2")
                nc.gpsimd.partition_broadcast(bc[0:DK, :], ra[:],
                                              channels=DK)
                nc.gpsimd.partition_broadcast(bc2[0:DK, :], rb[:],
                                              channels=DK)
                ot = otp.tile([P, QB], BF16, tag="ot", name="ot")
                nc.vector.tensor_mul(ot[0:DK, :], oa[0:DK, :], bc[0:DK, :])
                nc.vector.tensor_mul(ot[DK:P, :], ob[0:DK, :], bc2[0:DK, :])
                ot_tiles.append(ot)

            qsl = bass.ts(qb, QB)
            for n in range(NOC):
                yps = ps_b.tile([P, QB], F32, tag="sp", name="yps")
                for pr in range(NPAIR):
                    nc.tensor.matmul(
                        yps[:],
                        wo_sb[:, pr * D + n * P: pr * D + (n + 1) * P],
                        ot_tiles[pr][:],
                        start=(pr == 0), stop=(pr == NPAIR - 1))
                ysb = misc.tile([P, QB], F32, tag="ysb", name="ysb")
                nc.scalar.copy(ysb[:], yps[:])
                nc.sync.dma_start(yt_dram3[:, n, qsl], ysb[:])


_CACHE = {}


def _get_nc():
    if "nc" not in _CACHE:
        _CACHE["nc"] = build_bass()
    return _CACHE["nc"]


def host_prep(x, Wq, bq, Wk, bk, Wv, bv, Wo, bo):
    """Build the 8 per-core input maps."""
    NPAIR = HL // 2
    bf = ml_dtypes.bfloat16
    in_maps = []
    for core in range(N_CORES):
        b, g = divmod(core, 2)
        lo, hi = g * HL * DK, (g + 1) * HL * DK
        in_maps.append({
            "xT": np.ascontiguousarray(x[b].T).astype(bf),
            "Wq": np.ascontiguousarray(Wq[:, lo:hi]).astype(bf),
            "Wk": np.ascontiguousarray(Wk[:, lo:hi]).astype(bf),
            "Wv": np.ascontiguousarray(Wv[:, lo:hi]).astype(bf),
            "Wo": np.ascontiguousarray(Wo[lo:hi, :]).astype(bf),
            "bq_t": np.ascontiguousarray(bq[lo:hi].reshape(NPAIR, P).T),
            "bk_t": np.ascontiguousarray(bk[lo:hi].reshape(NPAIR, P).T),
        })
    return in_maps


def host_gather(results, bo):
    """Sum head-group partials, transpose back to [B, S, D], add bo."""
    out = np.empty((BATCH, SEQ, D_MODEL), dtype=np.float32)
    for b in range(BATCH):
        yt = results[2 * b]["YT"] + results[2 * b + 1]["YT"]
        out[b] = yt.T + bo
    return out


def kernel(x, Wq, bq, Wk, bk, Wv, bv, Wo, bo):
    nc = _get_nc()
    in_maps = host_prep(x, Wq, bq, Wk, bk, Wv, bv, Wo, bo)
    res = run_bass_kernel_spmd(nc, in_maps, core_ids=list(range(N_CORES)))
    # bv folds out of the device computation exactly: softmax rows sum to 1,
    # so P @ (V + 1 bv^T) = P @ V + 1 bv^T and the constant row bv @ Wo
    # joins the output bias.
    return host_gather(res.results, bo + bv @ Wo)
